# revision 45
# baseline (speedup 1.0000x reference)
"""GQA (B=2,S=1024,E=4096,H=32,KV=8,HD=128, RoPE, no causal mask) on 8 NeuronCores.

Sharding: 2 batch-groups x 4-way head tensor-parallel.
Core c: batch b=c//4, tp rank r=c%4 -> 8 q heads [8r,8r+8), 2 kv heads [2r,2r+2),
wo rows [1024r, 1024(r+1)).  Each core computes a partial output
out_part = y_local @ wo[local_rows, :]  (emitted transposed as [4096, 1024] fp16);
host sums the 4 partials per batch. No device collectives needed.

Schedule (v5, group-phased): the tensor engine only ever sees long stall-free
matmul streams so its DVFS p-state stays at max. The scalar engine's softmax
exp (the true attention pacer, ~1.07us per [128,1024] tile) is hidden under
those streams: scores for heads 0-2 ride the last projection superchunk,
head 3 rides attV of heads 0-1, and all of kv-group-1's scores ride the first
out-projection half-pass.  Phases:
  A: x@W projections (fp16, host-swizzled weights, contiguous DMAs)
  C: transpose V to natural layout (PE)
  attV(g0): heads 0-3, v as stationary weights -> yT directly in PSUM;
            softmax denominator = all-ones matmul (partition sum broadcast),
            fast approx reciprocal, scale on vector
  E1: out-projection contraction over heads 0-3 only, partials to SBUF fp16
  attV(g1): heads 4-7
  E2: contraction over heads 4-7, merged with E1 partials on vector
"""
import sys

sys.path.insert(0, "/opt/trn_rl_repo")

import numpy as np

B = 2
S = 1024
E = 4096
HD = 128
N_CORES = 8
TP = 4            # tensor-parallel ranks per batch group
HL = 8            # q heads per core
KVL = 2           # kv heads per core
QCOLS = HL * HD   # 1024
KVCOLS = KVL * HD  # 256
NCC = (QCOLS + 2 * KVCOLS) // 128  # 12 col-chunks of 128 (8 q, 2 k, 2 v)
ECH = E // 128    # 32 e-chunks
TT = S // 128     # 8 key tiles
ECS = 8           # e-chunks per superchunk
NSUP = ECH // ECS  # 4
GH = HL // 2      # heads per kv group (4)
SCALE = 1.0 / np.sqrt(np.float32(HD))
MM_DT = "float16"

_PROGRAM = None


def _build_program():
    import concourse.bass as bass  # noqa: F401
    from concourse import bacc
    import concourse.mybir as mybir
    from concourse.tile import TileContext
    from concourse.masks import make_identity

    dt = mybir.dt.float32
    dtr = getattr(mybir.dt, MM_DT)
    nc = bacc.Bacc("TRN2", target_bir_lowering=False, debug=False,
                   num_devices=N_CORES)

    xt_d = nc.declare_dram_parameter("xt", [E, S], dtr, isOutput=False)
    # host-swizzled qkv weights: row block (es*NCC+cc)*128+p, col ec*128+m
    #  = w_all[es*1024+ec*128+p, cc*128+m], w_all = concat(wq,wk,wv) cols
    wqkv_d = nc.declare_dram_parameter("wqkv", [NSUP * NCC * 128, ECS * 128],
                                       dtr, isOutput=False)
    # host-swizzled wo: row oc*128+p, col yc*128+i = wo[yc*128+p, oc*128+i]
    wo_d = nc.declare_dram_parameter("wo_p", [ECH * 128, HL * 128], dtr,
                                     isOutput=False)
    cos_d = nc.declare_dram_parameter("cos", [HD, S], dtr, isOutput=False)
    sinp_d = nc.declare_dram_parameter("sinp", [HD, S], dtr, isOutput=False)
    out_d = nc.declare_dram_parameter("out_t", [E, S], dtr, isOutput=True)

    # es<3 chunk order: k first (rope early), then q, then v
    CC_EARLY = [HL, HL + 1] + list(range(HL)) + [HL + KVL, HL + KVL + 1]
    # es=3: k, then q heads (scores chase the rope), v LAST so the final
    # psA drains are cheap plain adds and phase C starts promptly
    CC_LAST = [HL, HL + 1] + list(range(HL)) + [HL + KVL, HL + KVL + 1]

    # scores interleave for the es=3 superchunk: during chunk c's ec loop,
    # emit one scores kc-pair at ec in {1,3,5,7} for a head h with
    # 4+h+2 <= c (its rope is safely complete).  Covers heads 0..2.
    def build_es3_schedule():
        sched = {}
        pend = [(h, kc) for h in range(3) for kc in range(TT)]
        pi = 0
        for c in range(len(CC_LAST)):
            for ec in (1, 3, 5, 7):
                if pi >= len(pend):
                    break
                h, kc = pend[pi]
                if 2 + h + 2 <= c:
                    sched[(c, ec)] = (h, kc)
                    pi += 1
        assert pi == len(pend), (pi, len(pend))
        return sched

    ES3_SCHED = build_es3_schedule()

    with TileContext(nc) as tc:
        with tc.tile_pool(name="const", bufs=1) as cpool, \
             tc.tile_pool(name="persist", bufs=1) as ppool, \
             tc.tile_pool(name="vnat", bufs=1) as vpool:
            ident_f = cpool.tile([128, 128], dt)
            make_identity(nc, ident_f[:])
            ident = cpool.tile([128, 128], dtr)
            nc.vector.tensor_copy(ident[:], ident_f[:])
            ones_f = cpool.tile([128, 128], dt)
            nc.vector.memset(ones_f[:], 1.0)
            ones = cpool.tile([128, 128], dtr)
            nc.vector.tensor_copy(ones[:], ones_f[:])
            cos_t = cpool.tile([HD, S], dtr, tag="cos")
            sinp_t = cpool.tile([HD, S], dtr, tag="sinp")
            qkvT = [ppool.tile([128, S], dtr, tag=f"qkvT{i}", name=f"qkvT{i}")
                    for i in range(NCC)]
            yT = [ppool.tile([128, S], dtr, tag=f"yT{i}", name=f"yT{i}")
                  for i in range(HL)]
            v_nat = [[vpool.tile([128, HD], dtr, tag=f"v{kv}_{kt}",
                                 name=f"v{kv}_{kt}")
                      for kt in range(TT)] for kv in range(KVL)]

            pts = {}    # (h, kc) -> sbuf [128, S] fp16 exp'd scores
            tfs = {}    # h -> kc-sum tile
            yps = {}    # h -> psum yT accumulator

            def w_src(es, cc):
                base = (es * NCC + cc) * 128
                return wqkv_d[base:base + 128, :]

            def emit_scores_pair(h, kc, psS, ptspool, split=False):
                kv = h // GH
                kT = qkvT[HL + kv]
                pt = ptspool.tile([128, S], dtr, tag="pts")
                if split:
                    # two 1-bank psum tiles + two exps (E1: frees banks
                    # so the out-projection ring can go 3 deep)
                    for tb in range(2):
                        sp = psS.tile([128, 512], dt, tag="sp5")
                        nc.tensor.matmul(
                            sp[:],
                            kT[:, kc * 128:(kc + 1) * 128],
                            qkvT[h][:, tb * 512:(tb + 1) * 512],
                            start=True, stop=True)
                        nc.scalar.activation(
                            pt[:, tb * 512:(tb + 1) * 512], sp[:],
                            mybir.ActivationFunctionType.Exp,
                            scale=float(SCALE))
                else:
                    sp = psS.tile([128, S], dt, tag="sp")
                    for tb in range(2):
                        nc.tensor.matmul(
                            sp[:, tb * 512:(tb + 1) * 512],
                            kT[:, kc * 128:(kc + 1) * 128],
                            qkvT[h][:, tb * 512:(tb + 1) * 512],
                            start=True, stop=True)
                    nc.scalar.activation(pt[:], sp[:],
                                         mybir.ActivationFunctionType.Exp,
                                         scale=float(SCALE))
                pts[(h, kc)] = pt

            def emit_tree(h, tpool):
                # kc-sum of exp'd scores on the vector engine (7 adds)
                t0 = tpool.tile([128, S], dtr, tag="t0")
                t1 = tpool.tile([128, S], dtr, tag="t1")
                nc.vector.tensor_add(t0[:], pts[(h, 0)][:], pts[(h, 1)][:])
                nc.vector.tensor_add(t1[:], pts[(h, 2)][:], pts[(h, 3)][:])
                nc.vector.tensor_add(t0[:], t0[:], t1[:])
                t1 = tpool.tile([128, S], dtr, tag="t1")
                nc.vector.tensor_add(t1[:], pts[(h, 4)][:], pts[(h, 5)][:])
                nc.vector.tensor_add(t0[:], t0[:], t1[:])
                t1 = tpool.tile([128, S], dtr, tag="t1")
                nc.vector.tensor_add(t1[:], pts[(h, 6)][:], pts[(h, 7)][:])
                nc.vector.tensor_add(t0[:], t0[:], t1[:])
                tfs[h] = t0

            def emit_norm(h, psS, recpool, dntag="sp"):
                # all-ones matmul: partition-sum of the kc-sum broadcast to
                # all 128 partitions; approx reciprocal; scale yT
                dn = psS.tile([128, S], dt, tag=dntag)
                for tb in range(2):
                    nc.tensor.matmul(dn[:, tb * 512:(tb + 1) * 512], ones[:],
                                     tfs[h][:, tb * 512:(tb + 1) * 512],
                                     start=True, stop=True)
                rc = recpool.tile([128, S], dt, tag="rc")
                nc.vector.reciprocal_approx_fast(rc[:], dn[:])
                nc.vector.tensor_mul(yT[h][:], yps[h][:], rc[:])

            def emit_attv(h, psY, psS, ptspool, pair_slots):
                kv = h // GH
                yp = psY.tile([128, S], dt, tag="yp")
                yps[h] = yp
                for kc in range(TT):
                    for tb in range(2):
                        nc.tensor.matmul(
                            yp[:, tb * 512:(tb + 1) * 512],
                            v_nat[kv][kc][:],
                            pts[(h, kc)][:, tb * 512:(tb + 1) * 512],
                            start=(kc == 0), stop=(kc == TT - 1))
                    if kc in pair_slots:
                        emit_scores_pair(*pair_slots[kc], psS, ptspool)

            # ---------------- Phase A + interleaved scores ----------------
            with tc.tile_pool(name="wo", bufs=3) as wopool:
                ptspool_cm = tc.tile_pool(name="pts", bufs=32)
                ptspool = ptspool_cm.__enter__()
                psS_cm = tc.tile_pool(name="psS", bufs=2, space="PSUM")
                psS = psS_cm.__enter__()
                tpool_cm = tc.tile_pool(name="tsum2", bufs=1)
                tpool = tpool_cm.__enter__()
                # t0 ring needs 5 slots (3 live in g0 window, 4 in g1)
                t0pool_cm = tc.tile_pool(name="t0s", bufs=5)
                t0pool = t0pool_cm.__enter__()
                recpool_cm = tc.tile_pool(name="recs", bufs=2)
                recpool = recpool_cm.__enter__()

                class TP:  # route t0 and t1 tags to their own pools
                    _n = [0]

                    @staticmethod
                    def tile(shape, dtype, tag):
                        TP._n[0] += 1
                        pool = t0pool if tag == "t0" else tpool
                        return pool.tile(shape, dtype, tag=tag,
                                         name=f"ts{TP._n[0]}")

                with tc.tile_pool(name="xsup", bufs=2) as xspool, \
                     tc.tile_pool(name="wstream", bufs=3) as wpool, \
                     tc.tile_pool(name="rope", bufs=2) as ropool, \
                     tc.tile_pool(name="psA", bufs=2, space="PSUM") as psA:
                    for es in range(NSUP):
                        order = CC_LAST if es == NSUP - 1 else CC_EARLY
                        xs = xspool.tile([128, ECS, S], dtr, tag="xs",
                                         name=f"xs{es}")
                        if es == 0:
                            nc.sync.dma_start(out=xs[:, 0, :],
                                              in_=xt_d[0:128, :])
                            wt0 = wpool.tile([128, ECS * 128], dtr, tag="w",
                                             name="wt0_0")
                            nc.scalar.dma_start(out=wt0[:],
                                                in_=w_src(0, order[0]))
                            for ec in range(1, ECS):
                                nc.sync.dma_start(
                                    out=xs[:, ec, :],
                                    in_=xt_d[ec * 128:(ec + 1) * 128, :])
                        else:
                            for ec in range(ECS):
                                base = es * 1024 + ec * 128
                                nc.sync.dma_start(
                                    out=xs[:, ec, :],
                                    in_=xt_d[base:base + 128, :])
                            wt0 = wpool.tile([128, ECS * 128], dtr, tag="w",
                                             name=f"wt0_{es}")
                            nc.sync.dma_start(out=wt0[:],
                                              in_=w_src(es, order[0]))
                        if es == 1:
                            nc.sync.dma_start(out=cos_t[:], in_=cos_d[:])
                            nc.sync.dma_start(out=sinp_t[:], in_=sinp_d[:])
                        for ci, cc in enumerate(order):
                            if ci == 0:
                                wt = wt0
                            else:
                                wt = wpool.tile([128, ECS * 128], dtr, tag="w")
                                nc.sync.dma_start(out=wt[:], in_=w_src(es, cc))
                            acc = psA.tile([128, S], dt, tag="acc")
                            for ec in range(ECS):
                                for tb in range(2):
                                    nc.tensor.matmul(
                                        acc[:, tb * 512:(tb + 1) * 512],
                                        wt[:, ec * 128:(ec + 1) * 128],
                                        xs[:, ec, tb * 512:(tb + 1) * 512],
                                        start=(ec == 0), stop=(ec == ECS - 1))
                                if es == NSUP - 1 and (ci, ec) in ES3_SCHED:
                                    h, kc = ES3_SCHED[(ci, ec)]
                                    emit_scores_pair(h, kc, psS, ptspool)
                                    if kc == TT - 1:
                                        emit_tree(h, TP)
                            if es == 0:
                                nc.vector.tensor_copy(qkvT[cc][:], acc[:])
                            elif es < NSUP - 1 or cc >= HL + KVL:
                                nc.vector.tensor_add(qkvT[cc][:], acc[:],
                                                     qkvT[cc][:])
                            else:
                                # final accumulation + rope (q and k chunks)
                                rt = ropool.tile([128, S], dtr, tag="rt")
                                nc.vector.tensor_add(rt[:], acc[:], qkvT[cc][:])
                                sh = ropool.tile([HD, S], dtr, tag="sh")
                                nc.sync.dma_start(out=sh[0:64, :],
                                                  in_=rt[64:128, :])
                                nc.sync.dma_start(out=sh[64:128, :],
                                                  in_=rt[0:64, :])
                                t1 = ropool.tile([HD, S], dtr, tag="t1")
                                nc.vector.tensor_mul(t1[:], rt[:], cos_t[:])
                                nc.vector.tensor_mul(sh[:], sh[:], sinp_t[:])
                                nc.vector.tensor_add(qkvT[cc][:], t1[:], sh[:])

                # ---------------- Phase C: V natural tiles ----------------
                # head 3's first scores ride between the transposes
                with tc.tile_pool(name="psC", bufs=2, space="PSUM") as psC:
                    tcount = 0
                    for kv in range(KVL):
                        for kt in range(TT):
                            pt = psC.tile([128, 128], dtr, tag="ptC")
                            nc.tensor.transpose(
                                pt[:],
                                qkvT[HL + KVL + kv][:, kt * 128:(kt + 1) * 128],
                                ident[:])
                            nc.vector.tensor_copy(v_nat[kv][kt][:], pt[:])
                            tcount += 1
                            if tcount % 4 == 0:
                                emit_scores_pair(3, tcount // 4 - 1, psS,
                                                 ptspool)

                # prefetch first wo tiles
                wo_tiles = {}
                for oc in range(2):
                    wt = wopool.tile([128, HL * 128], dtr, tag="wo",
                                     name=f"wo{oc}")
                    nc.sync.dma_start(out=wt[:],
                                      in_=wo_d[oc * 128:(oc + 1) * 128, :])
                    wo_tiles[oc] = wt

                def get_wo(oc):
                    if oc in wo_tiles:
                        return wo_tiles.pop(oc)
                    wt = wopool.tile([128, HL * 128], dtr, tag="wo")
                    nc.sync.dma_start(out=wt[:],
                                      in_=wo_d[oc * 128:(oc + 1) * 128, :])
                    return wt

                # ---------------- attV all heads ----------------
                # pairs for head h+3ish ride attV(h) at half rate (always
                # slower than the scalar engine's exp drain -> no PE stalls,
                # p-state stays at max); late heads go full rate once the
                # exp queue has drained.
                half = lambda q: {1: q[0], 3: q[1], 5: q[2], 7: q[3]}
                six = lambda q: {0: q[0], 1: q[1], 3: q[2], 4: q[3],
                                 6: q[4], 7: q[5]}
                SLOTS = {
                    0: half([(3, 4), (3, 5), (3, 6), (3, 7)]),
                    1: half([(4, 0), (4, 1), (4, 2), (4, 3)]),
                    2: half([(4, 4), (4, 5), (4, 6), (4, 7)]),
                    3: six([(5, 0), (5, 1), (5, 2), (5, 3), (5, 4), (5, 5)]),
                    4: six([(5, 6), (5, 7), (6, 0), (6, 1), (6, 2), (6, 3)]),
                    5: six([(6, 4), (6, 5), (6, 6), (6, 7), (7, 0), (7, 1)]),
                    6: six([(7, 2), (7, 3), (7, 4), (7, 5), (7, 6), (7, 7)]),
                    7: {},
                }
                # trees for heads 0-2 were already emitted in es3; emit
                # each remaining tree as soon as its last scores pair is in
                # (uses early-D vector slack instead of piling onto the
                # saturated late-D periods)
                TREE_AFTER = {0: 3, 2: 4, 4: 5, 5: 6, 6: 7}
                # denominator + reciprocal for head h only need its score
                # tree, so they run ahead of the NEXT attV cluster; the yT
                # scale is emitted before the next tree so the psY slot
                # recycles without queuing behind 4.8us of tree-adds
                rcs = {}

                def emit_dnrec(h):
                    dn = psS.tile([128, S], dt, tag="sp")
                    for tb in range(2):
                        nc.tensor.matmul(dn[:, tb * 512:(tb + 1) * 512],
                                         ones[:],
                                         tfs[h][:, tb * 512:(tb + 1) * 512],
                                         start=True, stop=True)
                    rc = recpool.tile([128, S], dt, tag="rc")
                    nc.vector.reciprocal_approx_fast(rc[:], dn[:])
                    rcs[h] = rc

                def emit_mul(h):
                    nc.vector.tensor_mul(yT[h][:], yps[h][:], rcs[h][:])

                with tc.tile_pool(name="psY", bufs=2, space="PSUM") as psY:
                    for h in range(HL):
                        if h >= 1:
                            emit_dnrec(h - 1)
                        if h == HL - 1:
                            emit_dnrec(HL - 1)
                        emit_attv(h, psY, psS, ptspool, SLOTS[h])
                        if h >= 1:
                            emit_mul(h - 1)
                        if h in TREE_AFTER:
                            emit_tree(TREE_AFTER[h], TP)
                    emit_mul(HL - 1)

                # ---------------- E: out projection (single pass) ----------
                with tc.tile_pool(name="osb", bufs=3) as opool, \
                     tc.tile_pool(name="psO", bufs=2, space="PSUM") as psO:
                    # first two chains: run yc 0-6 for both before either
                    # touches yc7, so the PE has 28 matmuls of runway while
                    # head 7's final scale lands
                    stash = []
                    for oc in range(2):
                        wt = get_wo(oc)
                        op = psO.tile([128, S], dt, tag="op")
                        for yc in range(HL - 1):
                            for tb in range(2):
                                nc.tensor.matmul(
                                    op[:, tb * 512:(tb + 1) * 512],
                                    wt[:, yc * 128:(yc + 1) * 128],
                                    yT[yc][:, tb * 512:(tb + 1) * 512],
                                    start=(yc == 0), stop=False)
                        stash.append((oc, wt, op))
                    for oc, wt, op in stash:
                        yc = HL - 1
                        for tb in range(2):
                            nc.tensor.matmul(
                                op[:, tb * 512:(tb + 1) * 512],
                                wt[:, yc * 128:(yc + 1) * 128],
                                yT[yc][:, tb * 512:(tb + 1) * 512],
                                start=False, stop=True)
                        ot = opool.tile([128, S], dtr, tag="ot")
                        nc.scalar.copy(ot[:], op[:])
                        nc.sync.dma_start(
                            out=out_d[oc * 128:(oc + 1) * 128, :],
                            in_=ot[:])
                    for oc in range(2, ECH):
                        wt = get_wo(oc)
                        op = psO.tile([128, S], dt, tag="op")
                        for yc in range(HL):
                            for tb in range(2):
                                nc.tensor.matmul(
                                    op[:, tb * 512:(tb + 1) * 512],
                                    wt[:, yc * 128:(yc + 1) * 128],
                                    yT[yc][:, tb * 512:(tb + 1) * 512],
                                    start=(yc == 0), stop=(yc == HL - 1))
                        ot = opool.tile([128, S], dtr, tag="ot")
                        if oc >= ECH - 2:
                            for tb in range(2):
                                nc.scalar.copy(ot[:, tb * 512:(tb + 1) * 512],
                                               op[:, tb * 512:(tb + 1) * 512])
                                nc.sync.dma_start(
                                    out=out_d[oc * 128:(oc + 1) * 128,
                                              tb * 512:(tb + 1) * 512],
                                    in_=ot[:, tb * 512:(tb + 1) * 512])
                        else:
                            nc.scalar.copy(ot[:], op[:])
                            nc.sync.dma_start(
                                out=out_d[oc * 128:(oc + 1) * 128, :],
                                in_=ot[:])

                psS_cm.__exit__(None, None, None)
                recpool_cm.__exit__(None, None, None)
                t0pool_cm.__exit__(None, None, None)
                tpool_cm.__exit__(None, None, None)
                ptspool_cm.__exit__(None, None, None)

    nc.compile()
    return nc


def _rope_tables():
    inv = 1.0 / (10000.0 ** (np.arange(0, HD, 2, dtype=np.float32) / HD))  # [64]
    ang = np.arange(S, dtype=np.float32)[None, :] * inv[:, None]           # [64, S]
    cos = np.concatenate([np.cos(ang), np.cos(ang)], axis=0).astype(np.float32)
    sin = np.sin(ang)
    sinp = np.concatenate([-sin, sin], axis=0).astype(np.float32)          # [128, S]
    return cos, sinp


def make_in_maps(x, wq, wk, wv, wo):
    ndt = np.float16 if MM_DT == "float16" else np.float32
    cos, sinp = _rope_tables()
    cos = cos.astype(ndt)
    sinp = sinp.astype(ndt)
    x = np.ascontiguousarray(x, dtype=np.float32)
    xt = [np.ascontiguousarray(x[b].T).astype(ndt) for b in range(B)]

    wqkv_r, wo_r = [], []
    for r in range(TP):
        w_all = np.concatenate([
            wq[:, r * QCOLS:(r + 1) * QCOLS],
            wk[:, r * KVCOLS:(r + 1) * KVCOLS],
            wv[:, r * KVCOLS:(r + 1) * KVCOLS]], axis=1).astype(ndt)
        blocks = w_all.reshape(NSUP, ECS, 128, NCC, 128)   # es, ec, p, cc, m
        wqkv_r.append(np.ascontiguousarray(
            blocks.transpose(0, 3, 2, 1, 4).reshape(NSUP * NCC * 128,
                                                    ECS * 128)))
        wol = wo[r * QCOLS:(r + 1) * QCOLS, :].astype(ndt)
        t = wol.reshape(HL, 128, ECH, 128)                 # yc, p, oc, i
        wo_r.append(np.ascontiguousarray(
            t.transpose(2, 1, 0, 3).reshape(ECH * 128, HL * 128)))

    in_maps = []
    for c in range(N_CORES):
        b, r = c // TP, c % TP
        in_maps.append({
            "xt": xt[b],
            "wqkv": wqkv_r[r],
            "wo_p": wo_r[r],
            "cos": cos,
            "sinp": sinp,
        })
    return in_maps


def kernel(x, wq, wk, wv, wo):
    global _PROGRAM
    from concourse.bass_utils import run_bass_kernel_spmd

    if _PROGRAM is None:
        _PROGRAM = _build_program()
    nc = _PROGRAM

    in_maps = make_in_maps(x, wq, wk, wv, wo)
    res = run_bass_kernel_spmd(nc, in_maps, list(range(N_CORES)))

    out = np.zeros((B, S, E), dtype=np.float32)
    for c in range(N_CORES):
        b = c // TP
        out[b] += res.results[c]["out_t"].T.astype(np.float32)
    return out


# revision 47
# speedup vs baseline: 1.0038x; 1.0038x over previous
"""GQA (B=2,S=1024,E=4096,H=32,KV=8,HD=128, RoPE, no causal mask) on 8 NeuronCores.

Sharding: 2 batch-groups x 4-way head tensor-parallel.
Core c: batch b=c//4, tp rank r=c%4 -> 8 q heads [8r,8r+8), 2 kv heads [2r,2r+2),
wo rows [1024r, 1024(r+1)).  Each core computes a partial output
out_part = y_local @ wo[local_rows, :]  (emitted transposed as [4096, 1024] fp16);
host sums the 4 partials per batch. No device collectives needed.

Schedule (v5, group-phased): the tensor engine only ever sees long stall-free
matmul streams so its DVFS p-state stays at max. The scalar engine's softmax
exp (the true attention pacer, ~1.07us per [128,1024] tile) is hidden under
those streams: scores for heads 0-2 ride the last projection superchunk,
head 3 rides attV of heads 0-1, and all of kv-group-1's scores ride the first
out-projection half-pass.  Phases:
  A: x@W projections (fp16, host-swizzled weights, contiguous DMAs)
  C: transpose V to natural layout (PE)
  attV(g0): heads 0-3, v as stationary weights -> yT directly in PSUM;
            softmax denominator = all-ones matmul (partition sum broadcast),
            fast approx reciprocal, scale on vector
  E1: out-projection contraction over heads 0-3 only, partials to SBUF fp16
  attV(g1): heads 4-7
  E2: contraction over heads 4-7, merged with E1 partials on vector
"""
import sys

sys.path.insert(0, "/opt/trn_rl_repo")

import numpy as np

B = 2
S = 1024
E = 4096
HD = 128
N_CORES = 8
TP = 4            # tensor-parallel ranks per batch group
HL = 8            # q heads per core
KVL = 2           # kv heads per core
QCOLS = HL * HD   # 1024
KVCOLS = KVL * HD  # 256
NCC = (QCOLS + 2 * KVCOLS) // 128  # 12 col-chunks of 128 (8 q, 2 k, 2 v)
ECH = E // 128    # 32 e-chunks
TT = S // 128     # 8 key tiles
ECS = 8           # e-chunks per superchunk
NSUP = ECH // ECS  # 4
GH = HL // 2      # heads per kv group (4)
SCALE = 1.0 / np.sqrt(np.float32(HD))
MM_DT = "float16"

_PROGRAM = None


def _build_program():
    import concourse.bass as bass  # noqa: F401
    from concourse import bacc
    import concourse.mybir as mybir
    from concourse.tile import TileContext
    from concourse.masks import make_identity

    dt = mybir.dt.float32
    dtr = getattr(mybir.dt, MM_DT)
    nc = bacc.Bacc("TRN2", target_bir_lowering=False, debug=False,
                   num_devices=N_CORES)

    xt_d = nc.declare_dram_parameter("xt", [E, S], dtr, isOutput=False)
    # host-swizzled qkv weights: row block (es*NCC+cc)*128+p, col ec*128+m
    #  = w_all[es*1024+ec*128+p, cc*128+m], w_all = concat(wq,wk,wv) cols
    wqkv_d = nc.declare_dram_parameter("wqkv", [NSUP * NCC * 128, ECS * 128],
                                       dtr, isOutput=False)
    # host-swizzled wo: row oc*128+p, col yc*128+i = wo[yc*128+p, oc*128+i]
    wo_d = nc.declare_dram_parameter("wo_p", [ECH * 128, HL * 128], dtr,
                                     isOutput=False)
    cos_d = nc.declare_dram_parameter("cos", [HD, S], dtr, isOutput=False)
    sinp_d = nc.declare_dram_parameter("sinp", [HD, S], dtr, isOutput=False)
    out_d = nc.declare_dram_parameter("out_t", [E, S], dtr, isOutput=True)

    # es<3 chunk order: k first (rope early), then q, then v
    CC_EARLY = [HL, HL + 1] + list(range(HL)) + [HL + KVL, HL + KVL + 1]
    # es=3: k, then q heads (scores chase the rope), v LAST so the final
    # psA drains are cheap plain adds and phase C starts promptly
    CC_LAST = [HL, HL + 1] + list(range(HL)) + [HL + KVL, HL + KVL + 1]

    # scores interleave for the es=3 superchunk: during chunk c's ec loop,
    # emit one scores kc-pair at ec in {1,3,5,7} for a head h with
    # 4+h+2 <= c (its rope is safely complete).  Covers heads 0..2.
    def build_es3_schedule():
        sched = {}
        pend = [(h, kc) for h in range(3) for kc in range(TT)]
        pi = 0
        for c in range(len(CC_LAST)):
            for ec in (1, 3, 5, 7):
                if pi >= len(pend):
                    break
                h, kc = pend[pi]
                if 2 + h + 2 <= c:
                    sched[(c, ec)] = (h, kc)
                    pi += 1
        assert pi == len(pend), (pi, len(pend))
        return sched

    ES3_SCHED = build_es3_schedule()

    with TileContext(nc) as tc:
        with tc.tile_pool(name="const", bufs=1) as cpool, \
             tc.tile_pool(name="persist", bufs=1) as ppool, \
             tc.tile_pool(name="vnat", bufs=1) as vpool:
            ident_f = cpool.tile([128, 128], dt)
            make_identity(nc, ident_f[:])
            ident = cpool.tile([128, 128], dtr)
            nc.vector.tensor_copy(ident[:], ident_f[:])
            ones_f = cpool.tile([128, 128], dt)
            nc.vector.memset(ones_f[:], 1.0)
            ones = cpool.tile([128, 128], dtr)
            nc.vector.tensor_copy(ones[:], ones_f[:])
            cos_t = cpool.tile([HD, S], dtr, tag="cos")
            sinp_t = cpool.tile([HD, S], dtr, tag="sinp")
            qkvT = [ppool.tile([128, S], dtr, tag=f"qkvT{i}", name=f"qkvT{i}")
                    for i in range(NCC)]
            yT = [ppool.tile([128, S], dtr, tag=f"yT{i}", name=f"yT{i}")
                  for i in range(HL)]
            v_nat = [[vpool.tile([128, HD], dtr, tag=f"v{kv}_{kt}",
                                 name=f"v{kv}_{kt}")
                      for kt in range(TT)] for kv in range(KVL)]

            pts = {}    # (h, kc) -> sbuf [128, S] fp16 exp'd scores
            tfs = {}    # h -> kc-sum tile
            yps = {}    # h -> psum yT accumulator

            def w_src(es, cc):
                base = (es * NCC + cc) * 128
                return wqkv_d[base:base + 128, :]

            def emit_scores_pair(h, kc, psS, ptspool, split=False):
                kv = h // GH
                kT = qkvT[HL + kv]
                pt = ptspool.tile([128, S], dtr, tag="pts")
                if split:
                    # two 1-bank psum tiles + two exps (E1: frees banks
                    # so the out-projection ring can go 3 deep)
                    for tb in range(2):
                        sp = psS.tile([128, 512], dt, tag="sp5")
                        nc.tensor.matmul(
                            sp[:],
                            kT[:, kc * 128:(kc + 1) * 128],
                            qkvT[h][:, tb * 512:(tb + 1) * 512],
                            start=True, stop=True)
                        nc.scalar.activation(
                            pt[:, tb * 512:(tb + 1) * 512], sp[:],
                            mybir.ActivationFunctionType.Exp,
                            scale=float(SCALE))
                else:
                    sp = psS.tile([128, S], dt, tag="sp")
                    for tb in range(2):
                        nc.tensor.matmul(
                            sp[:, tb * 512:(tb + 1) * 512],
                            kT[:, kc * 128:(kc + 1) * 128],
                            qkvT[h][:, tb * 512:(tb + 1) * 512],
                            start=True, stop=True)
                    nc.scalar.activation(pt[:], sp[:],
                                         mybir.ActivationFunctionType.Exp,
                                         scale=float(SCALE))
                pts[(h, kc)] = pt

            def emit_tree(h, tpool):
                # kc-sum of exp'd scores on the vector engine (7 adds)
                t0 = tpool.tile([128, S], dtr, tag="t0")
                t1 = tpool.tile([128, S], dtr, tag="t1")
                nc.vector.tensor_add(t0[:], pts[(h, 0)][:], pts[(h, 1)][:])
                nc.vector.tensor_add(t1[:], pts[(h, 2)][:], pts[(h, 3)][:])
                nc.vector.tensor_add(t0[:], t0[:], t1[:])
                t1 = tpool.tile([128, S], dtr, tag="t1")
                nc.vector.tensor_add(t1[:], pts[(h, 4)][:], pts[(h, 5)][:])
                nc.vector.tensor_add(t0[:], t0[:], t1[:])
                t1 = tpool.tile([128, S], dtr, tag="t1")
                nc.vector.tensor_add(t1[:], pts[(h, 6)][:], pts[(h, 7)][:])
                nc.vector.tensor_add(t0[:], t0[:], t1[:])
                tfs[h] = t0

            def emit_norm(h, psS, recpool, dntag="sp"):
                # all-ones matmul: partition-sum of the kc-sum broadcast to
                # all 128 partitions; approx reciprocal; scale yT
                dn = psS.tile([128, S], dt, tag=dntag)
                for tb in range(2):
                    nc.tensor.matmul(dn[:, tb * 512:(tb + 1) * 512], ones[:],
                                     tfs[h][:, tb * 512:(tb + 1) * 512],
                                     start=True, stop=True)
                rc = recpool.tile([128, S], dt, tag="rc")
                nc.vector.reciprocal_approx_fast(rc[:], dn[:])
                nc.vector.tensor_mul(yT[h][:], yps[h][:], rc[:])

            def emit_attv(h, psY, psS, ptspool, pair_slots):
                kv = h // GH
                yp = psY.tile([128, S], dt, tag="yp")
                yps[h] = yp
                for kc in range(TT):
                    for tb in range(2):
                        nc.tensor.matmul(
                            yp[:, tb * 512:(tb + 1) * 512],
                            v_nat[kv][kc][:],
                            pts[(h, kc)][:, tb * 512:(tb + 1) * 512],
                            start=(kc == 0), stop=(kc == TT - 1))
                    if kc in pair_slots:
                        emit_scores_pair(*pair_slots[kc], psS, ptspool)

            # ---------------- Phase A + interleaved scores ----------------
            with tc.tile_pool(name="wo", bufs=3) as wopool:
                ptspool_cm = tc.tile_pool(name="pts", bufs=32)
                ptspool = ptspool_cm.__enter__()
                psS_cm = tc.tile_pool(name="psS", bufs=2, space="PSUM")
                psS = psS_cm.__enter__()
                tpool_cm = tc.tile_pool(name="tsum2", bufs=1)
                tpool = tpool_cm.__enter__()
                # t0 ring needs 5 slots (3 live in g0 window, 4 in g1)
                t0pool_cm = tc.tile_pool(name="t0s", bufs=5)
                t0pool = t0pool_cm.__enter__()
                recpool_cm = tc.tile_pool(name="recs", bufs=2)
                recpool = recpool_cm.__enter__()

                class TP:  # route t0 and t1 tags to their own pools
                    _n = [0]

                    @staticmethod
                    def tile(shape, dtype, tag):
                        TP._n[0] += 1
                        pool = t0pool if tag == "t0" else tpool
                        return pool.tile(shape, dtype, tag=tag,
                                         name=f"ts{TP._n[0]}")

                with tc.tile_pool(name="xsup", bufs=2) as xspool, \
                     tc.tile_pool(name="wstream", bufs=3) as wpool, \
                     tc.tile_pool(name="rope", bufs=2) as ropool, \
                     tc.tile_pool(name="psA", bufs=2, space="PSUM") as psA:
                    for es in range(NSUP):
                        order = CC_LAST if es == NSUP - 1 else CC_EARLY
                        xs = xspool.tile([128, ECS, S], dtr, tag="xs",
                                         name=f"xs{es}")
                        if es == 0:
                            nc.sync.dma_start(out=xs[:, 0, :],
                                              in_=xt_d[0:128, :])
                            wt0 = wpool.tile([128, ECS * 128], dtr, tag="w",
                                             name="wt0_0")
                            nc.scalar.dma_start(out=wt0[:],
                                                in_=w_src(0, order[0]))
                            for ec in range(1, ECS):
                                nc.sync.dma_start(
                                    out=xs[:, ec, :],
                                    in_=xt_d[ec * 128:(ec + 1) * 128, :])
                        else:
                            for ec in range(ECS):
                                base = es * 1024 + ec * 128
                                nc.sync.dma_start(
                                    out=xs[:, ec, :],
                                    in_=xt_d[base:base + 128, :])
                            wt0 = wpool.tile([128, ECS * 128], dtr, tag="w",
                                             name=f"wt0_{es}")
                            nc.sync.dma_start(out=wt0[:],
                                              in_=w_src(es, order[0]))
                        if es == 1:
                            nc.sync.dma_start(out=cos_t[:], in_=cos_d[:])
                            nc.sync.dma_start(out=sinp_t[:], in_=sinp_d[:])
                        for ci, cc in enumerate(order):
                            if ci == 0:
                                wt = wt0
                            else:
                                wt = wpool.tile([128, ECS * 128], dtr, tag="w")
                                nc.sync.dma_start(out=wt[:], in_=w_src(es, cc))
                            acc = psA.tile([128, S], dt, tag="acc")
                            for ec in range(ECS):
                                for tb in range(2):
                                    nc.tensor.matmul(
                                        acc[:, tb * 512:(tb + 1) * 512],
                                        wt[:, ec * 128:(ec + 1) * 128],
                                        xs[:, ec, tb * 512:(tb + 1) * 512],
                                        start=(ec == 0), stop=(ec == ECS - 1))
                                if es == NSUP - 1 and (ci, ec) in ES3_SCHED:
                                    h, kc = ES3_SCHED[(ci, ec)]
                                    emit_scores_pair(h, kc, psS, ptspool)
                                    if kc == TT - 1:
                                        emit_tree(h, TP)
                            if es == 0:
                                nc.vector.tensor_copy(qkvT[cc][:], acc[:])
                            elif es < NSUP - 1 or cc >= HL + KVL:
                                nc.vector.tensor_add(qkvT[cc][:], acc[:],
                                                     qkvT[cc][:])
                            else:
                                # final accumulation + rope (q and k chunks)
                                rt = ropool.tile([128, S], dtr, tag="rt")
                                nc.vector.tensor_add(rt[:], acc[:], qkvT[cc][:])
                                sh = ropool.tile([HD, S], dtr, tag="sh")
                                nc.sync.dma_start(out=sh[0:64, :],
                                                  in_=rt[64:128, :])
                                nc.sync.dma_start(out=sh[64:128, :],
                                                  in_=rt[0:64, :])
                                t1 = ropool.tile([HD, S], dtr, tag="t1")
                                nc.vector.tensor_mul(t1[:], rt[:], cos_t[:])
                                nc.vector.tensor_mul(sh[:], sh[:], sinp_t[:])
                                nc.vector.tensor_add(qkvT[cc][:], t1[:], sh[:])

                # ---------------- Phase C: V natural tiles ----------------
                # head 3's first scores ride between the transposes
                with tc.tile_pool(name="psC", bufs=2, space="PSUM") as psC:
                    tcount = 0
                    for kv in range(KVL):
                        for kt in range(TT):
                            pt = psC.tile([128, 128], dtr, tag="ptC")
                            nc.tensor.transpose(
                                pt[:],
                                qkvT[HL + KVL + kv][:, kt * 128:(kt + 1) * 128],
                                ident[:])
                            nc.vector.tensor_copy(v_nat[kv][kt][:], pt[:])
                            tcount += 1
                            if tcount % 4 == 0:
                                emit_scores_pair(3, tcount // 4 - 1, psS,
                                                 ptspool)

                # prefetch first wo tiles
                wo_tiles = {}
                for oc in range(2):
                    wt = wopool.tile([128, HL * 128], dtr, tag="wo",
                                     name=f"wo{oc}")
                    nc.sync.dma_start(out=wt[:],
                                      in_=wo_d[oc * 128:(oc + 1) * 128, :])
                    wo_tiles[oc] = wt

                def get_wo(oc):
                    if oc in wo_tiles:
                        return wo_tiles.pop(oc)
                    wt = wopool.tile([128, HL * 128], dtr, tag="wo")
                    nc.sync.dma_start(out=wt[:],
                                      in_=wo_d[oc * 128:(oc + 1) * 128, :])
                    return wt

                # ---------------- attV all heads ----------------
                # pairs for head h+3ish ride attV(h) at half rate (always
                # slower than the scalar engine's exp drain -> no PE stalls,
                # p-state stays at max); late heads go full rate once the
                # exp queue has drained.
                half = lambda q: {1: q[0], 3: q[1], 5: q[2], 7: q[3]}
                six = lambda q: {0: q[0], 1: q[1], 3: q[2], 4: q[3],
                                 6: q[4], 7: q[5]}
                SLOTS = {
                    0: half([(3, 4), (3, 5), (3, 6), (3, 7)]),
                    1: six([(4, 0), (4, 1), (4, 2), (4, 3), (4, 4), (4, 5)]),
                    2: six([(4, 6), (4, 7), (5, 0), (5, 1), (5, 2), (5, 3)]),
                    3: six([(5, 4), (5, 5), (5, 6), (5, 7), (6, 0), (6, 1)]),
                    4: six([(6, 2), (6, 3), (6, 4), (6, 5), (6, 6), (6, 7)]),
                    5: six([(7, 0), (7, 1), (7, 2), (7, 3), (7, 4), (7, 5)]),
                    6: {0: (7, 6), 1: (7, 7)},
                    7: {},
                }
                # trees for heads 0-2 were already emitted in es3; emit
                # each remaining tree as soon as its last scores pair is in
                # (uses early-D vector slack instead of piling onto the
                # saturated late-D periods)
                TREE_AFTER = {0: 3, 2: 4, 4: 5, 5: 6, 6: 7}
                # denominator + reciprocal for head h only need its score
                # tree, so they run ahead of the NEXT attV cluster; the yT
                # scale is emitted before the next tree so the psY slot
                # recycles without queuing behind 4.8us of tree-adds
                rcs = {}

                def emit_dnrec(h):
                    dn = psS.tile([128, S], dt, tag="sp")
                    for tb in range(2):
                        nc.tensor.matmul(dn[:, tb * 512:(tb + 1) * 512],
                                         ones[:],
                                         tfs[h][:, tb * 512:(tb + 1) * 512],
                                         start=True, stop=True)
                    rc = recpool.tile([128, S], dt, tag="rc")
                    nc.vector.reciprocal_approx_fast(rc[:], dn[:])
                    rcs[h] = rc

                def emit_mul(h):
                    nc.vector.tensor_mul(yT[h][:], yps[h][:], rcs[h][:])

                with tc.tile_pool(name="psY", bufs=2, space="PSUM") as psY:
                    for h in range(HL):
                        if h >= 1:
                            emit_dnrec(h - 1)
                        if h == HL - 1:
                            emit_dnrec(HL - 1)
                        emit_attv(h, psY, psS, ptspool, SLOTS[h])
                        if h >= 1:
                            emit_mul(h - 1)
                        if h in TREE_AFTER:
                            emit_tree(TREE_AFTER[h], TP)
                    emit_mul(HL - 1)

                # ---------------- E: out projection (single pass) ----------
                with tc.tile_pool(name="osb", bufs=3) as opool, \
                     tc.tile_pool(name="psO", bufs=2, space="PSUM") as psO:
                    for oc in range(ECH):
                        wt = get_wo(oc)
                        op = psO.tile([128, S], dt, tag="op")
                        for yc in range(HL):
                            for tb in range(2):
                                nc.tensor.matmul(
                                    op[:, tb * 512:(tb + 1) * 512],
                                    wt[:, yc * 128:(yc + 1) * 128],
                                    yT[yc][:, tb * 512:(tb + 1) * 512],
                                    start=(yc == 0), stop=(yc == HL - 1))
                        ot = opool.tile([128, S], dtr, tag="ot")
                        if oc >= ECH - 2:
                            for tb in range(2):
                                nc.scalar.copy(ot[:, tb * 512:(tb + 1) * 512],
                                               op[:, tb * 512:(tb + 1) * 512])
                                nc.sync.dma_start(
                                    out=out_d[oc * 128:(oc + 1) * 128,
                                              tb * 512:(tb + 1) * 512],
                                    in_=ot[:, tb * 512:(tb + 1) * 512])
                        else:
                            nc.scalar.copy(ot[:], op[:])
                            nc.sync.dma_start(
                                out=out_d[oc * 128:(oc + 1) * 128, :],
                                in_=ot[:])

                psS_cm.__exit__(None, None, None)
                recpool_cm.__exit__(None, None, None)
                t0pool_cm.__exit__(None, None, None)
                tpool_cm.__exit__(None, None, None)
                ptspool_cm.__exit__(None, None, None)

    nc.compile()
    return nc


def _rope_tables():
    inv = 1.0 / (10000.0 ** (np.arange(0, HD, 2, dtype=np.float32) / HD))  # [64]
    ang = np.arange(S, dtype=np.float32)[None, :] * inv[:, None]           # [64, S]
    cos = np.concatenate([np.cos(ang), np.cos(ang)], axis=0).astype(np.float32)
    sin = np.sin(ang)
    sinp = np.concatenate([-sin, sin], axis=0).astype(np.float32)          # [128, S]
    return cos, sinp


def make_in_maps(x, wq, wk, wv, wo):
    ndt = np.float16 if MM_DT == "float16" else np.float32
    cos, sinp = _rope_tables()
    cos = cos.astype(ndt)
    sinp = sinp.astype(ndt)
    x = np.ascontiguousarray(x, dtype=np.float32)
    xt = [np.ascontiguousarray(x[b].T).astype(ndt) for b in range(B)]

    wqkv_r, wo_r = [], []
    for r in range(TP):
        w_all = np.concatenate([
            wq[:, r * QCOLS:(r + 1) * QCOLS],
            wk[:, r * KVCOLS:(r + 1) * KVCOLS],
            wv[:, r * KVCOLS:(r + 1) * KVCOLS]], axis=1).astype(ndt)
        blocks = w_all.reshape(NSUP, ECS, 128, NCC, 128)   # es, ec, p, cc, m
        wqkv_r.append(np.ascontiguousarray(
            blocks.transpose(0, 3, 2, 1, 4).reshape(NSUP * NCC * 128,
                                                    ECS * 128)))
        wol = wo[r * QCOLS:(r + 1) * QCOLS, :].astype(ndt)
        t = wol.reshape(HL, 128, ECH, 128)                 # yc, p, oc, i
        wo_r.append(np.ascontiguousarray(
            t.transpose(2, 1, 0, 3).reshape(ECH * 128, HL * 128)))

    in_maps = []
    for c in range(N_CORES):
        b, r = c // TP, c % TP
        in_maps.append({
            "xt": xt[b],
            "wqkv": wqkv_r[r],
            "wo_p": wo_r[r],
            "cos": cos,
            "sinp": sinp,
        })
    return in_maps


def kernel(x, wq, wk, wv, wo):
    global _PROGRAM
    from concourse.bass_utils import run_bass_kernel_spmd

    if _PROGRAM is None:
        _PROGRAM = _build_program()
    nc = _PROGRAM

    in_maps = make_in_maps(x, wq, wk, wv, wo)
    res = run_bass_kernel_spmd(nc, in_maps, list(range(N_CORES)))

    out = np.zeros((B, S, E), dtype=np.float32)
    for c in range(N_CORES):
        b = c // TP
        out[b] += res.results[c]["out_t"].T.astype(np.float32)
    return out
